# revision 3
# baseline (speedup 1.0000x reference)
"""Tensor-parallel llama-style attention (prefill) on 8 TRN2 NeuronCores.

bf16 version, PE-roofline-oriented schedule. All matmul operands are bf16
(1 cycle/row on the PE, LDWEIGHTS fully hidden under 512-col streams).
fp8 was evaluated and rejected: e4m3's ~3.6% per-element quantization
noise is multiplicative into the output (>2e-2 budget); bf16 sims ~4e-3.

Sharding: tensor-parallel over heads. Core c holds q-heads [4c, 4c+4),
kv-head c, the matching rows of wq/wk/wv, and columns [512c, 512c+512) of
wo. Each core computes a full-size partial of the output projection;
partials are summed on the host (the "all-reduce after wo").

Key scheduling ideas (vs the naive phase-sequential version):
  - Softmax row sums are NOT computed with ones-column matmuls on the PE
    (those cost ~89us of PE time and run slower than full matmuls).
    Instead the exp tiles are accumulated on the DVE (bf16, 2x mode) and
    reduced across partitions once per (qb,h) with gpsimd
    partition_all_reduce, which also broadcasts - the PE does zero
    sum work and no partition_broadcast is needed.
  - ACT exp costs 0.833ns/col - exactly as much as the scores+AV matmul
    passes combined - so with sums removed the attention phase would be
    ACT-bound. The fix: the output-projection (W) matmuls for query
    block qb-1 are interleaved into the attention groups of qb (one W
    matmul after each kt-pair group), so the PE does useful W work
    while ACT drains exp. Remaining W work runs as a bulk block after
    each attention block.
  - Scores are computed into [128, 2, 512] two-bank PSUM tiles (kt-pair
    groups, double-buffered = 4 banks) so exp runs as one wide ACT
    instruction per off-diagonal group (halves ACT instruction count).
  - Causal masking: only the 128x128 diagonal TRIANGLE blocks actually
    need element masking (everything below is unmasked, everything
    above is never computed thanks to query-narrowing) - one m_tri tile
    built on-chip with affine_select, added per diagonal tile on DVE.
  - Diagonal score/AV/exp tiles are narrowed to the visible query range
    [128j, 512); the skipped region is never read.
  - RoPE: head-dim basis permuted on the host (even components first,
    odd second) -> half-partition swap + mul/add vs cos/sin tables,
    reading the projection result straight from PSUM.
  - PSUM bank placement: the attention ps2 pool sits on the banks freed
    earliest by the P1 epilogue (V/K first), shrinking the P1->A stall.
  - Non-causal masks fall back to the legacy kernel (mask DMA'd).
"""

import math
import os
import sys
from collections import deque

sys.path.insert(0, "/opt/trn_rl_repo")

import numpy as np
import ml_dtypes

import concourse.bacc as bacc
import concourse.tile as tile
import concourse.mybir as mybir
from concourse import masks
from concourse import bass_isa
from concourse.bass_utils import run_bass_kernel_spmd

B, S, DIM = 2, 2048, 4096
TOK = B * S
NH, NKV, HD = 32, 8, 128
NCORES = 8
HQ = NH // NCORES            # 4 query heads per core
SCALE = 1.0 / math.sqrt(HD)
F32 = mybir.dt.float32
BF16 = mybir.dt.bfloat16
NP_BF = ml_dtypes.bfloat16
EXP = mybir.ActivationFunctionType.Exp
COPY = mybir.ActivationFunctionType.Copy
GE = mybir.AluOpType.is_ge

QB = 4          # q-blocks per batch (512 queries each)
QW = S // QB    # 512
KT = S // 128   # 16 k-tiles per batch
NJ = HQ + 2     # 6 projection output tiles: 4 Q heads, K, V


def _build_causal():
    nc = bacc.Bacc("TRN2", target_bir_lowering=False, debug=False)

    xT_d = nc.dram_tensor("xT", [DIM, TOK], BF16, kind="ExternalInput")
    w_d = nc.dram_tensor("wqkvT", [DIM, NJ * HD], BF16, kind="ExternalInput")
    wo_d = nc.dram_tensor("woT", [HQ * HD, DIM], BF16, kind="ExternalInput")
    cos_d = nc.dram_tensor("cosT", [HD, S], F32, kind="ExternalInput")
    sin_d = nc.dram_tensor("sinTs", [HD, S], F32, kind="ExternalInput")
    out_d = nc.dram_tensor("out_part", [TOK, DIM], BF16,
                           kind="ExternalOutput")

    xT = xT_d.ap().rearrange("(kt p) t -> p kt t", p=128)      # [128, 32, TOK]
    w_ap = w_d.ap().rearrange("(kt p) j -> p kt j", p=128)     # [128, 32, 768]
    wo_ap = wo_d.ap().rearrange("(dt p) m -> p dt m", p=128)   # [128, 4, DIM]
    out_v = out_d.ap().rearrange("(g p) m -> p g m", p=128)    # [128, 32, DIM]

    with tile.TileContext(nc) as tc:
        with (
            tc.tile_pool(name="const", bufs=1) as const_pool,
            tc.tile_pool(name="batch", bufs=1) as batch_pool,
            tc.tile_pool(name="kv", bufs=2) as kv_pool,
        ):
            wqkv = const_pool.tile([128, 32, NJ * HD], BF16)
            # finest pieces first so the very first matmuls can start:
            # k-tile 0's V columns, then the rest of k-tile 0, then bulk
            nc.scalar.dma_start(wqkv[:, 0:1, 5 * HD:6 * HD],
                                w_ap[:, 0:1, 5 * HD:6 * HD])
            nc.scalar.dma_start(wqkv[:, 0:1, 0:5 * HD],
                                w_ap[:, 0:1, 0:5 * HD])
            for c0w, c1w in ((1, 2), (2, 8), (8, 16), (16, 24), (24, 32)):
                nc.scalar.dma_start(wqkv[:, c0w:c1w, :], w_ap[:, c0w:c1w, :])
            ident = const_pool.tile([128, 128], F32)
            masks.make_identity(nc, ident[:])
            # triangle mask for the diagonal 128x128 blocks: keep where
            # query-col >= key-partition, else -1e9
            m_tri = const_pool.tile([128, 128], F32)
            nc.gpsimd.memset(m_tri[:], 0.0)
            nc.gpsimd.affine_select(
                out=m_tri[:], in_=m_tri[:], compare_op=GE, fill=-1e9,
                base=0, pattern=[[1, 128]], channel_multiplier=-1)
            # wo is needed only by the W phase (~200us in); emitted later
            wo_s = const_pool.tile([128, HQ, DIM], BF16)

            # per-batch SBUF-resident activations
            qh_s = batch_pool.tile([128, HQ, S], BF16)
            att_h = batch_pool.tile([128, HQ, S], BF16)

            with (
                tc.tile_pool(name="xt", bufs=3) as xt_pool,
                tc.tile_pool(name="cs", bufs=2) as cs_pool,
                tc.tile_pool(name="rope", bufs=2) as rope_pool,
                tc.tile_pool(name="vtmp", bufs=2) as vtmp_pool,
                tc.tile_pool(name="pT", bufs=3) as pT_pool,
                tc.tile_pool(name="accp", bufs=2) as acc_pool,
                tc.tile_pool(name="sums", bufs=2) as sums_pool,
                tc.tile_pool(name="rcpp", bufs=2) as rcp_pool,
                tc.tile_pool(name="osb", bufs=2) as osb_pool,
            ):
              for b in range(B):
                kT_s = kv_pool.tile([128, S], BF16, tag="kT")
                v_s = kv_pool.tile([128, KT, HD], BF16, tag="v")
                # ---------- P1: QKV projections + RoPE ----------
                with (
                    tc.tile_pool(name="p1ps", bufs=NJ, space="PSUM") as p1ps,
                    tc.tile_pool(name="trps", bufs=2, space="PSUM") as trps,
                ):
                    for tb in range(4):          # 512-token chunks
                        c0 = b * S + tb * 512
                        sl = slice(tb * 512, tb * 512 + 512)
                        pss = [p1ps.tile([128, 512], F32, tag="ps",
                                         name=f"ps{j}")
                               for j in range(NJ)]
                        for ks in range(4):      # k slices of 8 x-tiles
                            xt_c = xt_pool.tile([128, 8, 512], BF16, tag="xt")
                            if b == 0 and tb == 0 and ks == 0:
                                # finest first transfers at cold start
                                nc.sync.dma_start(
                                    xt_c[:, 0:1, :], xT[:, 0:1, c0:c0 + 512])
                                nc.sync.dma_start(
                                    xt_c[:, 1:2, :], xT[:, 1:2, c0:c0 + 512])
                                nc.sync.dma_start(
                                    xt_c[:, 2:8, :], xT[:, 2:8, c0:c0 + 512])
                            else:
                                nc.sync.dma_start(
                                    xt_c[:],
                                    xT[:, ks * 8:(ks + 1) * 8, c0:c0 + 512])
                            for j in (5, 4, 0, 1, 2, 3):
                                for k in range(8):
                                    nc.tensor.matmul(
                                        pss[j][:],
                                        wqkv[:, ks * 8 + k,
                                             j * HD:(j + 1) * HD],
                                        xt_c[:, k, :],
                                        start=(ks == 0 and k == 0),
                                        stop=(ks == 3 and k == 7))
                        # cos/sin after the chunk's x slabs on the queue
                        cos_c = cs_pool.tile([HD, 512], F32, tag="cos")
                        sin_c = cs_pool.tile([HD, 512], F32, tag="sin")
                        nc.sync.dma_start(cos_c[:], cos_d.ap()[:, sl])
                        nc.sync.dma_start(sin_c[:], sin_d.ap()[:, sl])
                        # epilogue: V first (frees banks 5,6,7 for ps2),
                        # then K (bank 4), then the Q heads
                        for j in (5, 4, 0, 1, 2, 3):
                            ps = pss[j]
                            if j < HQ + 1:
                                # RoPE: out = z*cos + swap64(z)*sin_signed
                                tmp = rope_pool.tile([128, 512], F32,
                                                     tag="tmp")
                                nc.vector.tensor_mul(
                                    tmp[0:64, :], ps[64:128, :],
                                    sin_c[0:64, :])
                                nc.vector.tensor_mul(
                                    tmp[64:128, :], ps[0:64, :],
                                    sin_c[64:128, :])
                                t2 = rope_pool.tile([128, 512], F32, tag="t2")
                                nc.vector.tensor_mul(t2[:], ps[:], cos_c[:])
                                if j < HQ:
                                    nc.vector.tensor_add(
                                        qh_s[:, j, sl], t2[:], tmp[:])
                                else:
                                    nc.vector.tensor_add(
                                        kT_s[:, sl], t2[:], tmp[:])
                            else:
                                # V: copy from PSUM, transpose to token-major
                                v_sb = vtmp_pool.tile([128, 512], F32)
                                nc.vector.tensor_copy(v_sb[:], ps[:])
                                for h2 in range(4):
                                    tp = trps.tile([128, 128], F32)
                                    nc.tensor.transpose(
                                        tp[:],
                                        v_sb[:, h2 * 128:(h2 + 1) * 128],
                                        ident[:])
                                    nc.vector.tensor_copy(
                                        v_s[:, tb * 4 + h2, :], tp[:])

                if b == 0:
                    # load wo now: scalar queue is free, W needs it later
                    for mc in range(4):
                        nc.scalar.dma_start(
                            wo_s[:, :, mc * 1024:(mc + 1) * 1024],
                            wo_ap[:, :, mc * 1024:(mc + 1) * 1024])

                # ---------- A + W interleaved ----------
                # pool open order fixes PSUM banks: o_ps 0-1, wps 2-3,
                # ps2 4-7 (freed earliest by the P1 epilogue: V, K, trps)
                with (
                    tc.tile_pool(name="ops", bufs=2, space="PSUM") as o_pool,
                    tc.tile_pool(name="wps", bufs=2, space="PSUM") as w_pool,
                    tc.tile_pool(name="ps2", bufs=2, space="PSUM") as s_pool,
                ):
                    # -- W micro-op machinery --
                    w_state = {}

                    def w_step(qb, mb, ti, d4, bulk):
                        tt = qb * 4 + ti
                        if d4 == 0 and ti == 0:
                            w_state["osb"] = osb_pool.tile(
                                [128, 4, 512], BF16, tag="osb", name="o_sb")
                        if d4 == 0:
                            w_state["psw"] = w_pool.tile(
                                [128, 512], F32, tag="psw", name="ps_w")
                        ps_w = w_state["psw"]
                        nc.tensor.matmul(
                            ps_w[:],
                            att_h[:, d4, tt * 128:(tt + 1) * 128],
                            wo_s[:, d4, mb * 512:(mb + 1) * 512],
                            start=(d4 == 0), stop=(d4 == HQ - 1))
                        if d4 == HQ - 1:
                            o_sb = w_state["osb"]
                            if bulk and ti % 2 == 1:
                                nc.scalar.activation(
                                    o_sb[:, ti, :], ps_w[:], COPY)
                            else:
                                nc.vector.tensor_copy(o_sb[:, ti, :], ps_w[:])
                            if ti == 3:
                                g0 = b * (S // 128) + qb * 4
                                nc.scalar.dma_start(
                                    out_v[:, g0:g0 + 4,
                                          mb * 512:(mb + 1) * 512],
                                    o_sb[:])

                    w_pend = deque()

                    def w_make(qb):
                        for mb in range(8):
                            for ti in range(4):
                                for d4 in range(HQ):
                                    w_pend.append((qb, mb, ti, d4))

                    def w_pop(n, bulk=False):
                        for _ in range(n):
                            if not w_pend:
                                return
                            qb, mb, ti, d4 = w_pend.popleft()
                            w_step(qb, mb, ti, d4, bulk)

                    # -- attention units --
                    for qb in range(QB):
                        kt0 = 4 * qb
                        nkt = kt0 + 4
                        ngr = nkt // 2
                        for h in range(HQ):
                            o_ps = o_pool.tile([128, QW], F32, tag="o")
                            acc = acc_pool.tile([128, QW], BF16, tag="acc")
                            prev = None
                            for g in range(ngr):
                                ps2 = s_pool.tile([128, 2, QW], F32,
                                                  tag="s2")
                                pT2 = pT_pool.tile([128, 2, QW], BF16,
                                                   tag="p2")
                                kts = (2 * g, 2 * g + 1)
                                w0s = []
                                for i, kt in enumerate(kts):
                                    t = kt - kt0
                                    w0 = 128 * t if t >= 0 else 0
                                    w0s.append(w0)
                                    nc.tensor.matmul(
                                        ps2[:, i, w0:],
                                        kT_s[:, kt * 128:(kt + 1) * 128],
                                        qh_s[:, h,
                                             qb * QW + w0:(qb + 1) * QW],
                                        start=True, stop=True)
                                diag = kts[0] >= kt0
                                if diag:
                                    for i, kt in enumerate(kts):
                                        t = kt - kt0
                                        nc.vector.tensor_add(
                                            ps2[:, i,
                                                128 * t:128 * t + 128],
                                            ps2[:, i,
                                                128 * t:128 * t + 128],
                                            m_tri[:])
                                    for i in range(2):
                                        w0 = w0s[i]
                                        nc.scalar.activation(
                                            pT2[:, i, w0:], ps2[:, i, w0:],
                                            EXP, bias=0.0, scale=SCALE)
                                else:
                                    nc.scalar.activation(
                                        pT2[:, :, :], ps2[:, :, :],
                                        EXP, bias=0.0, scale=SCALE)
                                if prev is not None:
                                    pv, pkts, pw0s = prev
                                    for i, kt in enumerate(pkts):
                                        w0 = pw0s[i]
                                        nc.tensor.matmul(
                                            o_ps[:, w0:], v_s[:, kt, :],
                                            pv[:, i, w0:],
                                            start=(kt == 0),
                                            stop=(kt == nkt - 1))
                                        if kt == 0:
                                            nc.vector.tensor_copy(
                                                acc[:], pv[:, 0, :])
                                        else:
                                            nc.vector.tensor_add(
                                                acc[:, w0:], acc[:, w0:],
                                                pv[:, i, w0:])
                                    w_pop(1)
                                prev = (pT2, kts, w0s)
                            # drain last group
                            pv, pkts, pw0s = prev
                            w_pop(2)
                            for i, kt in enumerate(pkts):
                                w0 = pw0s[i]
                                nc.tensor.matmul(
                                    o_ps[:, w0:], v_s[:, kt, :],
                                    pv[:, i, w0:],
                                    start=(kt == 0), stop=(kt == nkt - 1))
                                if kt == 0:
                                    nc.vector.tensor_copy(acc[:], pv[:, 0, :])
                                else:
                                    nc.vector.tensor_add(
                                        acc[:, w0:], acc[:, w0:],
                                        pv[:, i, w0:])
                            w_pop(1)
                            # normalize: partition sums via gpsimd
                            # all-reduce (also broadcasts), then 1/x, mul
                            sums_t = sums_pool.tile([128, QW], F32, tag="sm")
                            nc.gpsimd.partition_all_reduce(
                                sums_t[:], acc[:], 128,
                                bass_isa.ReduceOp.add)
                            rcp_t = rcp_pool.tile([128, QW], F32, tag="rc")
                            nc.vector.reciprocal_approx_fast(
                                rcp_t[:], sums_t[:])
                            nc.vector.tensor_mul(
                                att_h[:, h, qb * QW:(qb + 1) * QW],
                                o_ps[:], rcp_t[:])
                        # A(qb) done: drain the rest of W(qb-1) as bulk,
                        # then queue W(qb) for injection into A(qb+1)
                        w_pop(len(w_pend), bulk=True)
                        w_make(qb)
                    # final block: W(q3) bulk
                    w_pop(len(w_pend), bulk=True)

    nc.compile()
    return nc


def _build_legacy(causal: bool):
    """Baseline kernel (phase-sequential, PE ones-matmul row sums).

    Kept for the non-causal mask fallback."""
    nc = bacc.Bacc("TRN2", target_bir_lowering=False, debug=False)

    xT_d = nc.dram_tensor("xT", [DIM, TOK], BF16, kind="ExternalInput")
    w_d = nc.dram_tensor("wqkvT", [DIM, NJ * HD], BF16, kind="ExternalInput")
    wo_d = nc.dram_tensor("woT", [HQ * HD, DIM], BF16, kind="ExternalInput")
    cos_d = nc.dram_tensor("cosT", [HD, S], F32, kind="ExternalInput")
    sin_d = nc.dram_tensor("sinTs", [HD, S], F32, kind="ExternalInput")
    if not causal:
        mask_d = nc.dram_tensor("maskTd", [QB * KT, 128, QW], BF16,
                                kind="ExternalInput")
    out_d = nc.dram_tensor("out_part", [TOK, DIM], BF16,
                           kind="ExternalOutput")

    xT = xT_d.ap().rearrange("(kt p) t -> p kt t", p=128)
    w_ap = w_d.ap().rearrange("(kt p) j -> p kt j", p=128)
    wo_ap = wo_d.ap().rearrange("(dt p) m -> p dt m", p=128)
    out_v = out_d.ap().rearrange("(g p) m -> p g m", p=128)

    with tile.TileContext(nc) as tc:
        with (
            tc.tile_pool(name="const", bufs=1) as const_pool,
            tc.tile_pool(name="batch", bufs=1) as batch_pool,
            tc.tile_pool(name="kv", bufs=2) as kv_pool,
        ):
            wqkv = const_pool.tile([128, 32, NJ * HD], BF16)
            for c0w, c1w in ((0, 2), (2, 8), (8, 16), (16, 24), (24, 32)):
                nc.scalar.dma_start(wqkv[:, c0w:c1w, :], w_ap[:, c0w:c1w, :])
            wo_s = const_pool.tile([128, HQ, DIM], BF16)
            for mc in range(4):
                nc.scalar.dma_start(
                    wo_s[:, :, mc * 1024:(mc + 1) * 1024],
                    wo_ap[:, :, mc * 1024:(mc + 1) * 1024])
            ident = const_pool.tile([128, 128], F32)
            masks.make_identity(nc, ident[:])
            ones_f = const_pool.tile([128, 1], F32)
            nc.vector.memset(ones_f[:], 1.0)
            ones_col = const_pool.tile([128, 1], BF16)
            nc.vector.tensor_copy(ones_col[:], ones_f[:])

            qh_s = batch_pool.tile([128, HQ, S], BF16)
            att_h = batch_pool.tile([128, HQ, S], BF16)

            with (
                tc.tile_pool(name="xt", bufs=2) as xt_pool,
                tc.tile_pool(name="cs", bufs=2) as cs_pool,
                tc.tile_pool(name="rope", bufs=2) as rope_pool,
                tc.tile_pool(name="vtmp", bufs=2) as vtmp_pool,
                tc.tile_pool(name="mask", bufs=1) as mask_pool,
                tc.tile_pool(name="pT", bufs=6) as p_pool,
                tc.tile_pool(name="rcp", bufs=2) as r_pool,
                tc.tile_pool(name="osb", bufs=2) as osb_pool,
            ):
              for b in range(B):
                kT_s = kv_pool.tile([128, S], BF16, tag="kT")
                v_s = kv_pool.tile([128, KT, HD], BF16, tag="v")
                with (
                    tc.tile_pool(name="p1ps", bufs=NJ, space="PSUM") as p1ps,
                    tc.tile_pool(name="trps", bufs=2, space="PSUM") as trps,
                ):
                    for tb in range(4):
                        c0 = b * S + tb * 512
                        sl = slice(tb * 512, tb * 512 + 512)
                        cos_c = cs_pool.tile([HD, 512], F32, tag="cos")
                        sin_c = cs_pool.tile([HD, 512], F32, tag="sin")
                        nc.sync.dma_start(cos_c[:], cos_d.ap()[:, sl])
                        nc.sync.dma_start(sin_c[:], sin_d.ap()[:, sl])
                        pss = [p1ps.tile([128, 512], F32, tag="ps",
                                         name=f"ps{j}")
                               for j in range(NJ)]
                        for ks in range(4):
                            xt_c = xt_pool.tile([128, 8, 512], BF16, tag="xt")
                            if b == 0 and tb == 0 and ks == 0:
                                nc.sync.dma_start(
                                    xt_c[:, 0:2, :], xT[:, 0:2, c0:c0 + 512])
                                nc.sync.dma_start(
                                    xt_c[:, 2:8, :], xT[:, 2:8, c0:c0 + 512])
                            else:
                                nc.sync.dma_start(
                                    xt_c[:],
                                    xT[:, ks * 8:(ks + 1) * 8, c0:c0 + 512])
                            for j in (5, 4, 0, 1, 2, 3):
                                for k in range(8):
                                    nc.tensor.matmul(
                                        pss[j][:],
                                        wqkv[:, ks * 8 + k,
                                             j * HD:(j + 1) * HD],
                                        xt_c[:, k, :],
                                        start=(ks == 0 and k == 0),
                                        stop=(ks == 3 and k == 7))
                        for j in (5, 4, 0, 1, 2, 3):
                            ps = pss[j]
                            if j < HQ + 1:
                                tmp = rope_pool.tile([128, 512], F32,
                                                     tag="tmp")
                                nc.vector.tensor_mul(
                                    tmp[0:64, :], ps[64:128, :],
                                    sin_c[0:64, :])
                                nc.vector.tensor_mul(
                                    tmp[64:128, :], ps[0:64, :],
                                    sin_c[64:128, :])
                                t2 = rope_pool.tile([128, 512], F32, tag="t2")
                                nc.vector.tensor_mul(t2[:], ps[:], cos_c[:])
                                if j < HQ:
                                    nc.vector.tensor_add(
                                        qh_s[:, j, sl], t2[:], tmp[:])
                                else:
                                    nc.vector.tensor_add(
                                        kT_s[:, sl], t2[:], tmp[:])
                            else:
                                v_sb = vtmp_pool.tile([128, 512], F32)
                                nc.vector.tensor_copy(v_sb[:], ps[:])
                                for h2 in range(4):
                                    tp = trps.tile([128, 128], F32)
                                    nc.tensor.transpose(
                                        tp[:],
                                        v_sb[:, h2 * 128:(h2 + 1) * 128],
                                        ident[:])
                                    nc.vector.tensor_copy(
                                        v_s[:, tb * 4 + h2, :], tp[:])

                with (
                    tc.tile_pool(name="sps", bufs=5, space="PSUM") as sps,
                    tc.tile_pool(name="sums", bufs=1, space="PSUM") as sums_ps,
                    tc.tile_pool(name="ops", bufs=2, space="PSUM") as o_ps_pool,
                ):
                    for qb in range(QB):
                        m_s = mask_pool.tile([128, KT, QW], BF16)
                        nc.scalar.dma_start(
                            m_s[:],
                            mask_d.ap()[qb * KT:(qb + 1) * KT]
                            .rearrange("kt p q -> p kt q"))
                        nkt = KT
                        for h in range(HQ):
                            sum_ps = sums_ps.tile([1, QW], F32)
                            o_ps = o_ps_pool.tile([128, QW], F32)
                            pend = []
                            for kt in range(nkt):
                                w0 = 0
                                s_ps = sps.tile([128, QW], F32, tag="s_ps")
                                nc.tensor.matmul(
                                    s_ps[:, w0:],
                                    kT_s[:, kt * 128:(kt + 1) * 128],
                                    qh_s[:, h, qb * QW + w0:(qb + 1) * QW],
                                    start=True, stop=True)
                                nc.vector.tensor_add(
                                    s_ps[:], s_ps[:], m_s[:, kt, :])
                                pT = p_pool.tile([128, QW], BF16, tag="pT")
                                nc.scalar.activation(
                                    pT[:, w0:], s_ps[:, w0:], EXP, bias=0.0,
                                    scale=SCALE)
                                pend.append((pT, kt, w0))
                                if len(pend) > 2:
                                    pv, pkt, pw = pend.pop(0)
                                    nc.tensor.matmul(
                                        sum_ps[:, pw:], ones_col[:],
                                        pv[:, pw:],
                                        start=(pkt == 0), stop=False)
                                    nc.tensor.matmul(
                                        o_ps[:, pw:], v_s[:, pkt, :],
                                        pv[:, pw:],
                                        start=(pkt == 0), stop=False)
                            while pend:
                                pv, pkt, pw = pend.pop(0)
                                last = not pend
                                nc.tensor.matmul(
                                    sum_ps[:, pw:], ones_col[:], pv[:, pw:],
                                    start=(pkt == 0), stop=last)
                                nc.tensor.matmul(
                                    o_ps[:, pw:], v_s[:, pkt, :], pv[:, pw:],
                                    start=(pkt == 0), stop=last)
                            rcp = r_pool.tile([1, QW], F32, tag="rcp")
                            nc.vector.reciprocal_approx_fast(
                                rcp[:], sum_ps[:])
                            bc_sb = r_pool.tile([128, QW], F32, tag="bc")
                            nc.gpsimd.partition_broadcast(bc_sb[:], rcp[:])
                            nc.vector.tensor_mul(
                                att_h[:, h, qb * QW:(qb + 1) * QW],
                                o_ps[:], bc_sb[:])

                with (
                    tc.tile_pool(name="wps", bufs=5, space="PSUM") as wps,
                ):
                    for mb in range(8):
                        for tg in range(4):
                            o_sb = osb_pool.tile([128, 4, 512], BF16)
                            for ts in range(4):
                                tt = tg * 4 + ts
                                ps_w = wps.tile([128, 512], F32)
                                for d4 in range(HQ):
                                    nc.tensor.matmul(
                                        ps_w[:],
                                        att_h[:, d4, tt * 128:(tt + 1) * 128],
                                        wo_s[:, d4,
                                             mb * 512:(mb + 1) * 512],
                                        start=(d4 == 0), stop=(d4 == HQ - 1))
                                if ts % 2 == 0:
                                    nc.vector.tensor_copy(
                                        o_sb[:, ts, :], ps_w[:])
                                else:
                                    nc.scalar.activation(
                                        o_sb[:, ts, :], ps_w[:], COPY)
                            g0 = b * (S // 128) + tg * 4
                            nc.sync.dma_start(
                                out_v[:, g0:g0 + 4, mb * 512:(mb + 1) * 512],
                                o_sb[:])

    nc.compile()
    return nc


_CACHE = {}
LAST_EXEC_NS = None


def _get_nc(causal: bool):
    if causal not in _CACHE:
        _CACHE[causal] = _build_causal() if causal else _build_legacy(False)
    return _CACHE[causal]


def _host_prep(x, wq, wk, wv, wo, freqs_cos, freqs_sin, mask):
    perm = np.concatenate([np.arange(0, HD, 2), np.arange(1, HD, 2)])
    wq_p = wq.reshape(NH, HD, DIM)[:, perm, :].reshape(NH * HD, DIM)
    wk_p = wk.reshape(NKV, HD, DIM)[:, perm, :].reshape(NKV * HD, DIM)

    xT = np.ascontiguousarray(x.reshape(TOK, DIM).T).astype(NP_BF)

    cos = freqs_cos.T                     # [64, S]
    sin = freqs_sin.T
    cosT = np.ascontiguousarray(np.concatenate([cos, cos], 0))       # [128, S]
    sinTs = np.ascontiguousarray(np.concatenate([-sin, sin], 0))

    ref_mask = np.triu(np.full((S, S), -1e9, dtype=np.float32), k=1)
    causal = np.array_equal(mask, ref_mask)

    if not causal:
        maskT = np.ascontiguousarray(mask.T) / np.float32(SCALE)   # [k, q]
        maskTd = np.empty((QB * KT, 128, QW), dtype=NP_BF)
        for qb in range(QB):
            for j in range(KT):
                maskTd[qb * KT + j] = maskT[j * 128:(j + 1) * 128,
                                            qb * QW:(qb + 1) * QW]

    in_maps = []
    for c in range(NCORES):
        wqT = wq_p[c * HQ * HD:(c + 1) * HQ * HD, :].T          # [DIM, 512]
        wkT = wk_p[c * HD:(c + 1) * HD, :].T                    # [DIM, 128]
        wvT = wv[c * HD:(c + 1) * HD, :].T                      # [DIM, 128]
        wqkvT = np.ascontiguousarray(
            np.concatenate([wqT, wkT, wvT], 1)).astype(NP_BF)
        woT = np.ascontiguousarray(
            wo[:, c * HQ * HD:(c + 1) * HQ * HD].T).astype(NP_BF)
        m = {"xT": xT, "wqkvT": wqkvT, "woT": woT,
             "cosT": cosT, "sinTs": sinTs}
        if not causal:
            m["maskTd"] = maskTd
        in_maps.append(m)
    return causal, in_maps


def kernel(x, wq, wk, wv, wo, freqs_cos, freqs_sin, mask, start_pos):
    global LAST_EXEC_NS
    causal, in_maps = _host_prep(
        np.asarray(x, np.float32), np.asarray(wq, np.float32),
        np.asarray(wk, np.float32), np.asarray(wv, np.float32),
        np.asarray(wo, np.float32), np.asarray(freqs_cos, np.float32),
        np.asarray(freqs_sin, np.float32), np.asarray(mask, np.float32))

    nc = _get_nc(causal)
    res = run_bass_kernel_spmd(nc, in_maps, core_ids=list(range(NCORES)))
    LAST_EXEC_NS = res.exec_time_ns

    acc = res.results[0]["out_part"].astype(np.float64)
    for c in range(1, NCORES):
        acc += res.results[c]["out_part"].astype(np.float64)
    return acc.astype(np.float32).reshape(B, S, DIM)


if __name__ == "__main__":
    rng = np.random.default_rng(0)
    inputs = {
        "x": rng.standard_normal((B, S, DIM), dtype=np.float32),
        "wq": (rng.standard_normal((DIM, DIM), dtype=np.float32) * 0.02),
        "wk": (rng.standard_normal((NKV * HD, DIM), dtype=np.float32) * 0.02),
        "wv": (rng.standard_normal((NKV * HD, DIM), dtype=np.float32) * 0.02),
        "wo": (rng.standard_normal((DIM, DIM), dtype=np.float32) * 0.02),
        "freqs_cos": rng.random((S, HD // 2), dtype=np.float32),
        "freqs_sin": rng.random((S, HD // 2), dtype=np.float32),
        "mask": np.triu(np.full((S, S), -1e9, dtype=np.float32), k=1),
        "start_pos": 0,
    }
    out = kernel(**inputs)
    print("out", out.shape, out.dtype, float(np.abs(out).mean()))
